# revision 17
# baseline (speedup 1.0000x reference)
"""Trainium2 Bass kernel for nn_Decoder_Cross_Projector.

Computation: kv = node @ W + b  -> split K/V caches -> rotary-rotate K by
mass sin/cos -> [2, B, H, N, KEY].

Sharding (8 cores, tensor-parallel on the head axis): core i owns k-heads
[16i,16i+16) and v-heads [16i,16i+16), i.e. a [1024, 2048] column slice of W.
`node` is replicated (transposed on host so the contraction dim lands on SBUF
partitions). Each core runs an identical program on its slice; outputs are
re-assembled host-side. No collectives.

Per-core device program (Tile framework):
  - W slice + broadcast bias resident in SBUF; node^T streamed per 128-token
    block; fp32r (fp22-multiply, fp32-accumulate) matmuls at full PE rate:
    64 token blocks x 4 psum banks x 8 K-chunks = 2048 matmuls of
    [128,128]^T @ [128,512], ~232 ns apart (PE ~91% busy).
  - Each psum bank is evacuated by one narrow DVE bias-add; the K-head
    rotary runs as 1024-wide SBUF-only DVE ops (2x mode). sin/cos come from
    ACT Sin on angles range-reduced to [-pi, pi] via i32 conversion plus a
    rounding-mode-agnostic fold.
  - DMA transfers serialize globally (all 16 engines gang per transfer), so
    the prologue enqueues exactly what the first matmuls need first.
  - Results DMA to a token-major [8192, 32, 64] per-core layout
    (4 KB-contiguous runs per token); host reassembles the final shape.
"""

import math

import numpy as np

import concourse.bass as bass
import concourse.tile as tile
from concourse import mybir
from concourse.bass_utils import run_bass_kernel_spmd
from concourse.tile import ScopedClock
from bass_rust import VectorClock, SyncInfo
from concourse.tile_sem_assignment import N_PROCS

f32 = mybir.dt.float32
f32r = mybir.dt.float32r

# ---------------------------------------------------------------------------
# Workarounds for this walrus build: it encodes at most ONE semaphore wait
# per instruction ("Too many sync wait commands" in setupSyncWait).
# (1) Replace TileContext's end-of-context drain (which carries one wait per
#     logical proc) with a chain of single-wait drains.
# (2) After tracing, hoist extra waits from any multi-wait instruction onto
#     InstNoOp carriers inserted immediately before it on the same engine.
# Both preserve semantics exactly: waits execute on the same engine stream,
# in the same order, before the guarded instruction.
# ---------------------------------------------------------------------------


def _drain_and_barrier_chunked(self, tick_clock, wait_clock):
    gc = tick_clock.global_clock
    prev = VectorClock()
    emitted = False
    for p in range(N_PROCS):
        if not gc[p]:
            continue
        partial = prev.copy()
        partial.require_at_least(p, gc[p])
        inst = self.nc.sync.drain()
        wait_clock.add_sem_waits(
            inst.ins, ScopedClock({None: partial}), ScopedClock({None: prev})
        )
        prev = partial
        emitted = True
    if not emitted:
        self.nc.sync.drain()
    self.nc.all_engine_barrier()
    assert self.sems is not None
    popped = self.nc._tile_sem_poison_stack.pop()
    assert popped is self._sem_poison
    self.nc.clear_and_free_semaphores(list(self.sems.allocated().values()))
    self.nc.all_engine_barrier()


tile.TileContext._drain_and_barrier = _drain_and_barrier_chunked

_DMA_INSTS = {"InstDMACopy", "InstDMA", "InstDmaTransposeAnt"}


def _split_multi_waits(nc):
    n_split = 0
    for f in nc.m.functions:
        for bb in f.blocks:
            insts = bb.instructions
            out = []
            changed = False
            for inst in insts:
                si = inst.sync_info
                if si is not None and len(si.on_wait) > 1:
                    # Keep a DMA-queue flow-control wait (DMAHW*/DMASW*) on
                    # the instruction itself; hoist the rest onto carriers.
                    waits = sorted(
                        si.on_wait,
                        key=lambda w: ("DMAHW" in w.ant_name
                                       or "DMASW" in w.ant_name)
                        if type(inst).__name__ in _DMA_INSTS else False,
                    )
                    for w in waits[:-1]:
                        nop = mybir.InstNoOp(
                            name=f"{inst.name}_waitc{n_split}", ins=[], outs=[]
                        )
                        nop.engine = inst.engine
                        nop.sync_info = SyncInfo(on_wait=[w], on_update=[])
                        out.append(nop)
                        n_split += 1
                    inst.sync_info = SyncInfo(
                        on_wait=[waits[-1]], on_update=list(si.on_update)
                    )
                    changed = True
                out.append(inst)
            if changed:
                bb.instructions = out
    return n_split


# ---------------------------------------------------------------------------
# Problem constants (hardcoded per the contract)
# ---------------------------------------------------------------------------
N_CORES = 8
B, SEQ, HIDDEN = 4, 2048, 1024
NUM_LAYERS, REL_SIZE, KEY = 8, 16, 64
HALF = KEY // 2  # 32
H = REL_SIZE * NUM_LAYERS  # 128 heads per cache
T = B * SEQ  # 8192 tokens
HPC = 2 * H // N_CORES  # 32 head-slots per core (16 K + 16 V)
FPC = HPC * KEY  # 2048 output features per core
KC = HIDDEN // 128  # 8 contraction chunks
NF = FPC // 512  # 4 psum tiles per token block
PI = math.pi

LAST_EXEC_TIME_NS = None


def build_nc(n_mblk=T // 128, split_waits=True):
    nc = bass.Bass()
    nodeT = nc.dram_tensor("nodeT", [HIDDEN, T], f32r, kind="ExternalInput")
    w = nc.dram_tensor("w", [HIDDEN, FPC], f32r, kind="ExternalInput")
    biasb = nc.dram_tensor("biasb", [128, FPC], f32, kind="ExternalInput")
    massr = nc.dram_tensor("massr", [128, T // 128], f32, kind="ExternalInput")
    invf = nc.dram_tensor("invf", [128, HALF], f32, kind="ExternalInput")
    out = nc.dram_tensor("out", [T, HPC, KEY], f32, kind="ExternalOutput")

    HW = FPC // 2  # 1024: K-half / V-half width per core

    with tile.TileContext(nc) as tc:
        with tc.tile_pool(name="wpool", bufs=1) as wpool, \
             tc.tile_pool(name="cpool", bufs=1) as cpool, \
             tc.tile_pool(name="npool", bufs=5) as npool, \
             tc.tile_pool(name="opool", bufs=6) as opool, \
             tc.tile_pool(name="tpool", bufs=4) as tpool, \
             tc.tile_pool(name="scpool", bufs=3) as scpool, \
             tc.tile_pool(name="pspool", bufs=8, space="PSUM") as pspool:

            def load_nt(mi):
                t = npool.tile([128, KC, 128], f32r, tag="nt")
                nc.sync.dma_start(
                    t[:],
                    nodeT[:, mi * 128:(mi + 1) * 128].rearrange(
                        "(kc p) t -> p kc t", p=128))
                return t

            # DMA order matters: transfers serialize globally, so enqueue
            # what the first matmuls need first (K weights, first slab),
            # then the rest.
            def load_wcol(ci):
                t = wpool.tile([128, KC, 512], f32r, tag=f"w{ci}")
                nc.sync.dma_start(
                    t[:], w[:, ci * 512:(ci + 1) * 512].rearrange(
                        "(kc p) n -> p kc n", p=128))
                return t

            wcol = [None] * 4
            wcol[0] = load_wcol(0)
            invf_sb = cpool.tile([128, HALF], f32)
            nc.sync.dma_start(invf_sb[:], invf[:])
            massr_sb = cpool.tile([128, T // 128], f32)
            nc.sync.dma_start(massr_sb[:], massr[:])
            nts = {0: load_nt(0)}
            wcol[1] = load_wcol(1)
            biasK_sb = cpool.tile([128, HW], f32)
            nc.sync.dma_start(biasK_sb[:], biasb[:, 0:HW])
            wcol[2] = load_wcol(2)
            wcol[3] = load_wcol(3)
            biasV_sb = cpool.tile([128, HW], f32)
            nc.sync.dma_start(biasV_sb[:], biasb[:, HW:FPC])
            nts[1] = load_nt(1)
            # const AP for Sin bias (+pi/2, folds the cos shift into ACT)
            hpib = cpool.tile([128, 1], f32)
            nc.vector.memset(hpib[:], 0.5 * PI)

            for m in range(n_mblk):
                nt = nts.pop(m)
                if m + 2 < n_mblk:
                    nts[m + 2] = load_nt(m + 2)

                # --- angle + sin/cos, batched for 2 token blocks ---
                # HW Sin is only accurate for |x| <= pi. red = ang - 2pi*q
                # with q = i32(ang/2pi) (rounds-to-nearest on HW, truncates
                # in CoreSim), then a mode-agnostic fold (s>pi -> s-=2pi)
                # lands in [-pi, pi] either way. cos(ang) = sin(red + pi/2),
                # re-folded at pi/2 with the +pi/2 shift in the ACT bias.
                if m % 2 == 0:
                    nb = min(2, n_mblk - m)
                    mass2 = massr_sb[:, m:m + nb].unsqueeze(2).to_broadcast(
                        (128, nb, HALF))
                    invb = invf_sb[:].unsqueeze(1).to_broadcast(
                        (128, nb, HALF))
                    ang2 = scpool.tile([128, 2, HALF], f32, tag="ang2")
                    nc.vector.tensor_tensor(
                        ang2[:, :nb], mass2, invb, mybir.AluOpType.mult)
                    q2 = scpool.tile([128, 2, HALF], mybir.dt.int32, tag="q2")
                    nc.vector.tensor_scalar(
                        q2[:, :nb], ang2[:, :nb], 1.0 / (2.0 * PI), None,
                        mybir.AluOpType.mult)
                    qf2 = scpool.tile([128, 2, HALF], f32, tag="qf2")
                    nc.vector.tensor_copy(qf2[:, :nb], q2[:, :nb])
                    s12 = scpool.tile([128, 2, HALF], f32, tag="s12")
                    nc.vector.scalar_tensor_tensor(
                        s12[:, :nb], qf2[:, :nb], -2.0 * PI, ang2[:, :nb],
                        mybir.AluOpType.mult, mybir.AluOpType.add)
                    g12 = scpool.tile([128, 2, HALF], f32, tag="g12")
                    nc.vector.tensor_scalar(
                        g12[:, :nb], s12[:, :nb], PI, None,
                        mybir.AluOpType.is_gt)
                    red2 = scpool.tile([128, 2, HALF], f32, tag="red2")
                    nc.vector.scalar_tensor_tensor(
                        red2[:, :nb], g12[:, :nb], -2.0 * PI, s12[:, :nb],
                        mybir.AluOpType.mult, mybir.AluOpType.add)
                    gc2 = scpool.tile([128, 2, HALF], f32, tag="gc2")
                    nc.vector.tensor_scalar(
                        gc2[:, :nb], red2[:, :nb], 0.5 * PI, None,
                        mybir.AluOpType.is_gt)
                    redc2 = scpool.tile([128, 2, HALF], f32, tag="redc2")
                    nc.vector.scalar_tensor_tensor(
                        redc2[:, :nb], gc2[:, :nb], -2.0 * PI, red2[:, :nb],
                        mybir.AluOpType.mult, mybir.AluOpType.add)
                    # [p, blk, 0:32] = -sin, [p, blk, 32:64] = +sin
                    snsn2 = scpool.tile([128, 2, KEY], f32, tag="snsn2")
                    nc.scalar.activation(
                        snsn2[:, :nb, 0:HALF], red2[:, :nb],
                        mybir.ActivationFunctionType.Sin, scale=-1.0)
                    nc.scalar.activation(
                        snsn2[:, :nb, HALF:KEY], red2[:, :nb],
                        mybir.ActivationFunctionType.Sin)
                    cos2 = scpool.tile([128, 2, HALF], f32, tag="cos2")
                    nc.scalar.activation(
                        cos2[:, :nb], redc2[:, :nb],
                        mybir.ActivationFunctionType.Sin, bias=hpib[:])
                blk = m % 2
                cos_t = cos2[:, blk]
                snsn = snsn2[:, blk]

                # --- matmuls: four 1-bank psum tiles (best PE pipelining);
                # psum evacuation = narrow bias-adds; rotary = wide SBUF ops.
                for half_i in range(2):  # 0 = K heads, 1 = V heads
                    bias_sl = biasK_sb if half_i == 0 else biasV_sb
                    tt = tpool.tile([128, HW], f32, tag="tt")
                    for sub in range(2):
                        wc = wcol[half_i * 2 + sub]
                        ps = pspool.tile([128, 512], f32)
                        for kc in range(KC):
                            nc.tensor.matmul(
                                ps[:],
                                lhsT=nt[:, kc, :],
                                rhs=wc[:, kc, :],
                                start=(kc == 0), stop=(kc == KC - 1))
                        # evacuate promptly: bank free after this one op
                        nc.vector.tensor_tensor(
                            tt[:, sub * 512:(sub + 1) * 512], ps[:],
                            bias_sl[:, sub * 512:(sub + 1) * 512],
                            mybir.AluOpType.add)
                    if half_i == 0:
                        # K heads: rotary as 1024-wide SBUF-only ops (2x mode)
                        ob = opool.tile([128, HW], f32)
                        t3 = tt[:].rearrange("p (j h d) -> p j h d", j=16, h=2)
                        o3 = ob[:].rearrange("p (j h d) -> p j h d", j=16, h=2)
                        cosb = cos_t.unsqueeze(1).unsqueeze(2).to_broadcast(
                            (128, 16, 2, HALF))
                        nc.vector.tensor_tensor(
                            o3, t3, cosb, mybir.AluOpType.mult)
                        m2 = tpool.tile([128, HW], f32, tag="m2")
                        m23 = m2[:].rearrange(
                            "p (j h d) -> p j h d", j=16, h=2)
                        negs = snsn[:, 0:HALF].unsqueeze(1).to_broadcast(
                            (128, 16, HALF))
                        sins = snsn[:, HALF:KEY].unsqueeze(1).to_broadcast(
                            (128, 16, HALF))
                        nc.vector.tensor_tensor(
                            m23[:, :, 0, :], t3[:, :, 1, :], negs,
                            mybir.AluOpType.mult)
                        nc.vector.tensor_tensor(
                            m23[:, :, 1, :], t3[:, :, 0, :], sins,
                            mybir.AluOpType.mult)
                        nc.vector.tensor_tensor(
                            ob[:], ob[:], m2[:], mybir.AluOpType.add)
                        src = ob
                    else:
                        src = tt  # V heads: bias-added result is final
                    dst = out[m * 128:(m + 1) * 128,
                              half_i * 16:(half_i + 1) * 16, :]
                    nc.sync.dma_start(
                        dst, src[:].rearrange("p (j d) -> p j d", j=16))

    if split_waits:
        _split_multi_waits(nc)
    return nc


def prep_inputs(node, node_mass, W, b):
    """Host-side layout prep + per-core sharding."""
    node = np.ascontiguousarray(np.asarray(node, dtype=np.float32))
    node_mass = np.ascontiguousarray(np.asarray(node_mass, dtype=np.float32))
    W = np.ascontiguousarray(np.asarray(W, dtype=np.float32))
    b = np.ascontiguousarray(np.asarray(b, dtype=np.float32))

    nodeT = np.ascontiguousarray(node.reshape(T, HIDDEN).T)  # [1024, 8192]
    massr = np.ascontiguousarray(
        node_mass.reshape(T // 128, 128).T)  # [128, 64]
    inv_freq = np.exp(
        -np.log(np.float32(10000.0))
        * np.arange(HALF, dtype=np.float32) / np.float32(HALF)
    ).astype(np.float32)
    invf = np.ascontiguousarray(np.broadcast_to(inv_freq, (128, HALF)))

    in_maps = []
    for i in range(N_CORES):
        k_cols = slice(i * 1024, (i + 1) * 1024)
        v_cols = slice(H * KEY + i * 1024, H * KEY + (i + 1) * 1024)
        wi = np.ascontiguousarray(
            np.concatenate([W[:, k_cols], W[:, v_cols]], axis=1))
        bi = np.concatenate([b[k_cols], b[v_cols]])
        biasb = np.ascontiguousarray(
            np.broadcast_to(bi, (128, FPC)).astype(np.float32))
        in_maps.append({
            "nodeT": nodeT, "w": wi, "biasb": biasb,
            "massr": massr, "invf": invf,
        })
    return in_maps


_NC_CACHE = {}


def kernel(node, node_mass, W, b):
    global LAST_EXEC_TIME_NS
    if "nc" not in _NC_CACHE:
        _NC_CACHE["nc"] = build_nc()
    nc = _NC_CACHE["nc"]

    in_maps = prep_inputs(node, node_mass, W, b)
    res = run_bass_kernel_spmd(nc, in_maps, list(range(N_CORES)),
                               trace=False)
    LAST_EXEC_TIME_NS = res.exec_time_ns

    full = np.empty((2, B, H, SEQ, KEY), dtype=np.float32)
    for i in range(N_CORES):
        oc = res.results[i]["out"].reshape(B, SEQ, HPC, KEY)
        full[0, :, 16 * i:16 * (i + 1)] = oc[:, :, :16].transpose(0, 2, 1, 3)
        full[1, :, 16 * i:16 * (i + 1)] = oc[:, :, 16:].transpose(0, 2, 1, 3)
    return full
